# revision 9
# baseline (speedup 1.0000x reference)
"""Causal scaled-dot attention (softmax over query axis) for Trainium2.

Full inputs -> full outputs; internally data-parallel over batch across 8
NeuronCores (2 batches per core for B=16).

Math per batch b (S=2048, H=1024):
  q = queries @ Wq.T + bq ; k = keys @ Wk.T + bk ; v = values @ Wv.T + bv
  scores = q @ k.T * H**-0.5
  masked = scores + NEG * triu(ones)     # diagonal masked too
  attn = softmax(masked, axis=0 of q)    # normalized over the QUERY axis
  context = attn @ v
  returns (context, attn)

Device strategy: everything is computed transposed (attnT[j, i]) so the
softmax runs along the free dimension. TensorE transposes (via identity)
produce h-major copies of the inputs; projections/scores/context run as
fp32r matmuls (full PE rate, ~11-bit mantissa). The last attention column
(j = S-1) is fully masked: the reference's `scores - 1e7` quantizes scores
to a 1.0-spaced fp32 grid there, which amplifies fp32r noise by e^{+-1} -
so that one column is recomputed exactly through an fp32 path:
  B = Wk.T @ Wq (once), u = B @ keys[b,-1], s_col = queries @ u (DVE dots),
then softmaxed identically to the reference and injected as partition 127
of the final attnT row-block.
"""
import math
import numpy as np

import concourse.bass as bass
import concourse.tile as tile
from concourse import bacc, mybir
from concourse.bass_utils import run_bass_kernel_spmd
from concourse.masks import make_identity, make_lower_triangular

dt = mybir.dt
P = 128
NEG = -10000000.0
N_CORES = 8
TRACE = False  # test harness may flip this


def build(B_pc, S, H, n_cores):
    HT = H // P           # h/o partition tiles
    ST = S // P           # s partition tiles
    CW = min(512, S)      # scores chunk width (psum bank)
    NCH = S // CW
    CWH = min(512, H)     # H chunk width (context)
    HCH = H // CWH
    CWV = min(256, H)     # H chunk width (v-proj rhs stripes, SBUF-lean)
    HCV = H // CWV
    TG = 2                # input-transpose group (psum cols = TG*P)
    TG2 = 4               # attn-out transpose group

    f32, f32r = dt.float32, dt.float32r
    X = mybir.AxisListType.X
    AF = mybir.ActivationFunctionType

    nc = bacc.Bacc("TRN2", target_bir_lowering=False, debug=False,
                   enable_asserts=False, num_devices=n_cores)

    q_in = nc.dram_tensor("queries", (B_pc, S, H), f32, kind="ExternalInput").ap()
    k_in = nc.dram_tensor("keys", (B_pc, S, H), f32, kind="ExternalInput").ap()
    v_in = nc.dram_tensor("values", (B_pc, S, H), f32, kind="ExternalInput").ap()
    w_in, b_in = {}, {}
    for x in ("q", "k", "v"):
        w_in[x] = nc.dram_tensor(f"W{x}", (H, H), f32, kind="ExternalInput").ap()
        b_in[x] = nc.dram_tensor(f"b{x}", (H,), f32, kind="ExternalInput").ap()
    ctx_out = nc.dram_tensor("context", (B_pc, S, H), f32,
                             kind="ExternalOutput").ap()
    attn_out = nc.dram_tensor("attn", (B_pc, S, S), f32,
                              kind="ExternalOutput").ap()

    sc = 1.0 / math.sqrt(H)  # power of two for our shapes -> exact multiply

    with tile.TileContext(nc) as tc:
        with tc.tile_pool(name="big", bufs=1) as big, \
             tc.tile_pool(name="work", bufs=2) as work, \
             tc.tile_pool(name="dram", bufs=1, space="DRAM") as dpool, \
             tc.tile_pool(name="psA", bufs=2, space="PSUM") as psA, \
             tc.tile_pool(name="psT", bufs=2, space="PSUM") as psT, \
             tc.tile_pool(name="psC", bufs=3, space="PSUM") as psC:

            # ---------------- constants ----------------
            ident = work.tile([P, P], f32, tag="ident", bufs=1)
            make_identity(nc, ident)
            ident_r = work.tile([P, P], f32r, tag="identr", bufs=1)
            nc.vector.tensor_copy(ident_r[:], ident[:])
            tri = work.tile([P, P], f32, tag="tri", bufs=1)
            make_lower_triangular(nc, tri, val=NEG, diag=True)

            bq_sb = work.tile([P, HT], f32, tag="bq", bufs=1)
            nc.sync.dma_start(bq_sb[:], b_in["q"].rearrange("(t p) -> p t", p=P))
            bk_sb = work.tile([P, HT], f32, tag="bk", bufs=1)
            nc.sync.dma_start(bk_sb[:], b_in["k"].rearrange("(t p) -> p t", p=P))
            bv_bc = work.tile([P, H], f32, tag="bv", bufs=1)
            nc.sync.dma_start(bv_bc[:], b_in["v"][None].to_broadcast((P, H)))

            # ---------------- DRAM scratch ----------------
            wT_d = {
                "q": dpool.tile([HT, HT, P, P], f32r, name="wqT_d"),
                "k": dpool.tile([HT, HT, P, P], f32r, name="wkT_d"),
                "v": dpool.tile([HT, P, H], f32r, name="wvT_d"),
            }
            B_d = dpool.tile([H, H], f32, name="B_d")
            kT_d = [dpool.tile([ST, HT, P, P], f32r, name=f"kT_d{b}")
                    for b in range(B_pc)]
            v_d = [dpool.tile([S, H], f32r, name=f"v_d{b}")
                   for b in range(B_pc)]
            aT_d = [dpool.tile([ST, ST, P, P], f32r, name=f"aT_d{b}")
                    for b in range(B_pc)]
            u_d = [dpool.tile([H], f32, name=f"u_d{b}") for b in range(B_pc)]
            scol_d = [dpool.tile([S], f32, name=f"scol_d{b}")
                      for b in range(B_pc)]
            acol_d = [dpool.tile([S], f32r, name=f"acol_d{b}")
                      for b in range(B_pc)]

            def ph_tile():
                return work.tile([P, H], f32, tag="ph", bufs=4, name="ph")

            # ---------------- weight transposes (once) ----------------
            for x in ("q", "k", "v"):
                for ob in range(HT):
                    wsl = ph_tile()
                    nc.sync.dma_start(wsl[:], w_in[x][ob * P:(ob + 1) * P, :])
                    for g in range(HT // TG):
                        pst = psT.tile([P, TG2 * P], f32, tag="pst", name="pst")
                        for t in range(TG):
                            hb = g * TG + t
                            nc.tensor.transpose(
                                pst[:, t * P:(t + 1) * P],
                                wsl[:, hb * P:(hb + 1) * P], ident)
                        wst = work.tile([P, TG2 * P], f32r, tag="p5", name="wst")
                        nc.vector.tensor_copy(wst[:, :TG * P], pst[:, :TG * P])
                        for t in range(TG):
                            hb = g * TG + t
                            if x == "v":
                                nc.sync.dma_start(
                                    wT_d["v"][hb, :, ob * P:(ob + 1) * P],
                                    wst[:, t * P:(t + 1) * P])
                            else:
                                nc.sync.dma_start(
                                    wT_d[x][ob, hb],
                                    wst[:, t * P:(t + 1) * P])

            # ---------------- B = Wk^T @ Wq in fp32 (once) ----------------
            for ch in range(HCV):
                wqs = work.tile([P, HT, CWV], f32, tag="wvh", bufs=1, name="wqs")
                nc.sync.dma_start(
                    wqs[:], w_in["q"][:, ch * CWV:(ch + 1) * CWV]
                    .rearrange("(ob p) h -> p ob h", p=P))
                for hpb in range(HT):
                    wks = work.tile([P, HT, P], f32, tag="kslc", name="wks")
                    nc.sync.dma_start(
                        wks[:], w_in["k"][:, hpb * P:(hpb + 1) * P]
                        .rearrange("(ob p) h -> p ob h", p=P))
                    psb = psA.tile([P, CW], f32, tag="score", name="psb")
                    for ob in range(HT):
                        nc.tensor.matmul(psb[:, :CWV], wks[:, ob], wqs[:, ob],
                                         start=(ob == 0), stop=(ob == HT - 1))
                    bst = work.tile([P, TG2 * P], f32, tag="p5", name="bst")
                    nc.vector.tensor_copy(bst[:, :CWV], psb[:, :CWV])
                    nc.sync.dma_start(
                        B_d[hpb * P:(hpb + 1) * P, ch * CWV:(ch + 1) * CWV],
                        bst[:, :CWV])

            # ---------------- helpers ----------------
            def transpose_input(src2d, xT, fuse_scol=None):
                """src2d: DRAM [S, H] -> xT[:, hb, :] = src2d.T (f32r).
                fuse_scol: (ubc, scol_sb) to also emit per-slice dot products."""
                for g0 in range(0, ST, TG):
                    xs = []
                    for t in range(TG):
                        sl = ph_tile()
                        nc.sync.dma_start(
                            sl[:], src2d[(g0 + t) * P:(g0 + t + 1) * P, :])
                        xs.append(sl)
                    if fuse_scol is not None:
                        ubc, scol_sb = fuse_scol
                        for t in range(TG):
                            tmp = ph_tile()
                            nc.vector.tensor_mul(tmp[:], xs[t][:], ubc[:])
                            nc.vector.reduce_sum(
                                scol_sb[:, g0 + t:g0 + t + 1], tmp[:], axis=X)
                    for hb in range(HT):
                        pst = psT.tile([P, TG2 * P], f32, tag="pst", name="pst")
                        for t in range(TG):
                            nc.tensor.transpose(
                                pst[:, t * P:(t + 1) * P],
                                xs[t][:, hb * P:(hb + 1) * P], ident)
                        nc.vector.tensor_copy(
                            xT[:, hb, g0 * P:(g0 + TG) * P], pst[:, :TG * P])

            def project_T(xT, wkey, bias_sb, out_cb):
                """out[ob][:, s] = (W @ x.T)[ob-block] + bias, f32r.
                out_cb(ob, ch, psum) consumes each [P, CW] chunk."""
                for ob in range(HT):
                    wsl = work.tile([P, HT, P], f32r, tag="kslc", name="wsl")
                    nc.sync.dma_start(
                        wsl[:], wT_d[wkey][ob].rearrange("hb p o -> p hb o"))
                    for ch in range(NCH):
                        ps = psA.tile([P, CW], f32, tag="score", name="ps")
                        for hb in range(HT):
                            nc.tensor.matmul(
                                ps[:], wsl[:, hb],
                                xT[:, hb, ch * CW:(ch + 1) * CW],
                                start=(hb == 0), stop=(hb == HT - 1))
                        out_cb(ob, ch, ps)

            # ---------------- per-batch pipeline ----------------
            for b in range(B_pc):
                # ===== keys: transpose + projection -> kT_d =====
                xT = big.tile([P, HT, S], f32r, tag="xTvp", name="xTk")
                transpose_input(k_in[b], xT)

                krows = {}

                def k_cb(ob, ch, ps, krows=krows, b=b):
                    if ch == 0:
                        krows[ob] = work.tile([P, S], f32r, tag="rowr",
                                              name="krow")
                    nc.scalar.activation(
                        krows[ob][:, ch * CW:(ch + 1) * CW], ps[:],
                        AF.Identity, bias=bk_sb[:, ob:ob + 1])
                    if ch == NCH - 1:
                        for jb in range(ST):
                            nc.sync.dma_start(
                                kT_d[b][jb, ob],
                                krows[ob][:, jb * P:(jb + 1) * P])

                project_T(xT, "k", bk_sb, k_cb)

                # ===== last-column exact path: u = B @ k_lastT =====
                klastT = work.tile([P, HT], f32, tag="klastT", bufs=1,
                                   name="klastT")
                nc.sync.dma_start(
                    klastT[:], k_in[b, S - 1].rearrange("(hb p) -> p hb", p=P))
                u_sb = work.tile([P, HT], f32, tag="usb", bufs=1, name="u_sb")
                for hb in range(HT):
                    bsl = work.tile([P, HT, P], f32, tag="kslc", name="bsl")
                    nc.sync.dma_start(
                        bsl[:], B_d[:, hb * P:(hb + 1) * P]
                        .rearrange("(g p) h -> p g h", p=P))
                    psu = psA.tile([P, CW], f32, tag="score", name="psu")
                    for g in range(HT):
                        nc.tensor.matmul(psu[:, :1], bsl[:, g],
                                         klastT[:, g:g + 1],
                                         start=(g == 0), stop=(g == HT - 1))
                    nc.vector.tensor_copy(u_sb[:, hb:hb + 1], psu[:, :1])
                    nc.sync.dma_start(u_d[b][hb * P:(hb + 1) * P],
                                      u_sb[:, hb])

                ubc = ph_tile()
                nc.sync.dma_start(ubc[:], u_d[b][None].to_broadcast((P, H)))

                # ===== queries: transpose + s_col dots + projection =====
                xTq = big.tile([P, HT, S], f32r, tag="xTvp", name="xTq")
                scol_sb = work.tile([P, ST], f32, tag="scolp", bufs=1,
                                    name="scol_sb")
                transpose_input(q_in[b], xTq, fuse_scol=(ubc, scol_sb))
                nc.vector.tensor_scalar_mul(scol_sb[:], scol_sb[:], sc)
                nc.sync.dma_start(
                    scol_d[b].rearrange("(ib p) -> p ib", p=P), scol_sb[:])

                qT = big.tile([P, HT, S], f32r, tag="qT", name="qT")

                def q_cb(ob, ch, ps, qT=qT):
                    nc.scalar.activation(
                        qT[:, ob, ch * CW:(ch + 1) * CW], ps[:],
                        AF.Identity, bias=bq_sb[:, ob:ob + 1])

                project_T(xTq, "q", bq_sb, q_cb)

                # ===== s_col softmax (reference-exact masked column) =====
                srow = work.tile([P, S], f32, tag="xrow", name="srow")
                nc.sync.dma_start(srow[:1, :], scol_d[b][None, :])
                nc.vector.tensor_scalar_add(srow[:1, :], srow[:1, :], NEG)
                nmx0 = work.tile([P, 1], f32, tag="nmx", name="nmx0")
                nc.vector.reduce_max(nmx0[:1], srow[:1, :], axis=X, negate=True)
                rs0 = work.tile([P, 1], f32, tag="rs", name="rs0")
                nc.scalar.activation(srow[:1, :], srow[:1, :], AF.Exp,
                                     bias=nmx0[:1], accum_out=rs0[:1])
                ri0 = work.tile([P, 1], f32, tag="ri", name="ri0")
                nc.vector.reciprocal(ri0[:1], rs0[:1])
                acol = work.tile([P, S], f32r, tag="rowr", name="acol")
                nc.vector.tensor_scalar_mul(acol[:1, :], srow[:1, :], ri0[:1])
                nc.sync.dma_start(acol_d[b][None, :], acol[:1, :])

                # ===== values: transpose + projection -> v_d =====
                xTv = big.tile([P, HT, S], f32r, tag="xTvp", name="xTv")
                transpose_input(v_in[b], xTv)
                for hc in range(HCV):
                    wvh = work.tile([P, HT, CWV], f32r, tag="wvh", bufs=1,
                                    name="wvh")
                    nc.sync.dma_start(
                        wvh[:], wT_d["v"][:, :, hc * CWV:(hc + 1) * CWV]
                        .rearrange("hb p c -> p hb c"))
                    for sb in range(ST):
                        ps = psA.tile([P, CW], f32, tag="score", name="psv")
                        for hb in range(HT):
                            nc.tensor.matmul(
                                ps[:, :CWV], xTv[:, hb, sb * P:(sb + 1) * P],
                                wvh[:, hb],
                                start=(hb == 0), stop=(hb == HT - 1))
                        vst = work.tile([P, TG2 * P], f32r, tag="p5",
                                        name="vst")
                        nc.vector.tensor_add(
                            vst[:, :CWV], ps[:, :CWV],
                            bv_bc[:, hc * CWV:(hc + 1) * CWV])
                        nc.sync.dma_start(
                            v_d[b][sb * P:(sb + 1) * P,
                                   hc * CWV:(hc + 1) * CWV],
                            vst[:, :CWV])

                # ===== scores + column-softmax (rows of attnT) =====
                for jb in range(ST):
                    j0 = jb * P
                    ksl = work.tile([P, HT, P], f32r, tag="kslc", name="ksl")
                    nc.sync.dma_start(
                        ksl[:], kT_d[b][jb].rearrange("ob p j -> p ob j"))
                    xrow = work.tile([P, S], f32, tag="xrow", name="xrow")
                    for ch in range(NCH):
                        c0 = ch * CW
                        ps = psA.tile([P, CW], f32, tag="score", name="pss")
                        for ob in range(HT):
                            nc.tensor.matmul(
                                ps[:], ksl[:, ob], qT[:, ob, c0:c0 + CW],
                                start=(ob == 0), stop=(ob == HT - 1))
                        xch = xrow[:, c0:c0 + CW]
                        nc.scalar.mul(xch, ps[:], sc)
                        if c0 + CW <= j0:
                            nc.vector.tensor_scalar_add(xch, xch, NEG)
                        elif c0 >= j0 + P:
                            pass
                        else:
                            if j0 > c0:
                                nc.vector.tensor_scalar_add(
                                    xrow[:, c0:j0], xrow[:, c0:j0], NEG)
                            nc.vector.tensor_add(
                                xrow[:, j0:j0 + P], xrow[:, j0:j0 + P], tri[:])
                    nmx = work.tile([P, 1], f32, tag="nmx", name="nmx")
                    nc.vector.reduce_max(nmx[:], xrow[:], axis=X, negate=True)
                    rs = work.tile([P, 1], f32, tag="rs", name="rs")
                    nc.scalar.activation(xrow[:], xrow[:], AF.Exp,
                                         bias=nmx[:], accum_out=rs[:])
                    ri = work.tile([P, 1], f32, tag="ri", name="ri")
                    nc.vector.reciprocal(ri[:], rs[:])
                    arow = work.tile([P, S], f32r, tag="rowr", name="arow")
                    nc.vector.tensor_scalar_mul(arow[:], xrow[:], ri[:])
                    if jb == ST - 1:
                        # exact fully-masked column replaces partition 127
                        nc.sync.dma_start(arow[P - 1:P, :], acol_d[b][None, :])
                    for ib in range(ST):
                        nc.sync.dma_start(aT_d[b][jb, ib],
                                          arow[:, ib * P:(ib + 1) * P])
                    for g0 in range(0, ST, TG2):
                        pst = psT.tile([P, TG2 * P], f32r, tag="pst",
                                       name="psta")
                        for t in range(TG2):
                            ib = g0 + t
                            nc.tensor.transpose(
                                pst[:, t * P:(t + 1) * P],
                                arow[:, ib * P:(ib + 1) * P], ident_r)
                        ost = work.tile([P, TG2 * P], f32, tag="p5",
                                        name="ost")
                        nc.vector.tensor_copy(ost[:], pst[:])
                        for t in range(TG2):
                            ib = g0 + t
                            nc.sync.dma_start(
                                attn_out[b, ib * P:(ib + 1) * P, j0:j0 + P],
                                ost[:, t * P:(t + 1) * P])

                # ===== context = attnT.T @ v =====
                vp = big.tile([P, ST, H], f32r, tag="xTvp", name="vp")
                for jb in range(ST):
                    nc.sync.dma_start(vp[:, jb, :],
                                      v_d[b][jb * P:(jb + 1) * P, :])
                for ib in range(ST):
                    pcs = [psC.tile([P, CWH], f32, tag="ctx", name="pc")
                           for _ in range(HCH)]
                    for jb in range(ST):
                        abk = work.tile([P, P], f32r, tag="ablk", bufs=3,
                                        name="abk")
                        nc.sync.dma_start(abk[:], aT_d[b][jb, ib])
                        for hc in range(HCH):
                            nc.tensor.matmul(
                                pcs[hc][:], abk[:],
                                vp[:, jb, hc * CWH:(hc + 1) * CWH],
                                start=(jb == 0), stop=(jb == ST - 1))
                    cst = ph_tile()
                    for hc in range(HCH):
                        nc.scalar.copy(cst[:, hc * CWH:(hc + 1) * CWH],
                                       pcs[hc][:])
                    nc.sync.dma_start(ctx_out[b, ib * P:(ib + 1) * P, :],
                                      cst[:])

    nc.compile()
    return nc


_CACHE = {}


def _get_nc(B_pc, S, H, n_cores):
    key = (B_pc, S, H, n_cores)
    if key not in _CACHE:
        _CACHE[key] = build(*key)
    return _CACHE[key]


def kernel(queries, keys, values, Wq, bq, Wk, bk, Wv, bv):
    B, S, H = queries.shape
    n_cores = N_CORES
    B_pc = B // n_cores
    nc = _get_nc(B_pc, S, H, n_cores)

    def f(x):
        return np.ascontiguousarray(np.asarray(x, dtype=np.float32))

    shared = dict(Wq=f(Wq), bq=f(bq), Wk=f(Wk), bk=f(bk), Wv=f(Wv), bv=f(bv))
    qs, ks, vs = f(queries), f(keys), f(values)
    in_maps = []
    for c in range(n_cores):
        m = dict(shared)
        m["queries"] = qs[c * B_pc:(c + 1) * B_pc]
        m["keys"] = ks[c * B_pc:(c + 1) * B_pc]
        m["values"] = vs[c * B_pc:(c + 1) * B_pc]
        in_maps.append(m)
    res = run_bass_kernel_spmd(nc, in_maps, core_ids=list(range(n_cores)),
                               trace=TRACE)
    kernel.last_results = res
    context = np.concatenate([r["context"] for r in res.results], axis=0)
    attn = np.concatenate([r["attn"] for r in res.results], axis=0)
    return context, attn
